# revision 60
# baseline (speedup 1.0000x reference)
"""Bayes predictor (retrieval-kNN softmax) Trainium2 kernel, 8 NeuronCores.

Math (reference):
    logits[b,n] = -(0.5*D*log(var_b) + 0.5/var_b * ||inputs_b - sqrt(a_b)*data_n||^2)
    probs = softmax(logits, axis=n);  x0 = probs @ data
    out = (inputs - sqrt(a)*x0)/sqrt(var)

Per-row-b softmax is invariant to per-b constants, so with
    s1_b = sqrt(a_b)/var_b,  s2_b = -a_b/(2 var_b),  r_n = ||data_n||^2
we use  l[b,n] = s1_b*(inputs_b . data_n) + s2_b*r_n.

Sharding: data_batch split along N across 8 cores (4096 rows each).
Each core computes a partial (max, weighted-sum, sum) triple; one
AllGather + a local combine produces the full output on every core.

Per-core layout ("packed-j"): logits live in PSUM as [128, 2, 512]
where partition p = 32*j + b covers n-group j (j in 0..3, 1024 n per
group), so vector/scalar ops run at full 128-partition width.

Schedule notes (why this is fast):
- w16 rides in the dt tensor (one fewer descriptor); identities are
  synthesized on-chip; cbc's descriptor generation runs on the
  Pool/SWDGE path in parallel with the SP/HWDGE dt descriptors.
- PE p-state warm-up fillers hold the ramp near full speed for mm1.
- l_1 accumulates its s2*r term FIRST so the last data matmul closes
  the bank: the critical tail after the last dt piece is 2 matmuls.
- E^T PSUM->SBUF copies run on DVE (2x perf mode beats Pool there).
- The j-combine and the cross-core combine are trees split across the
  DVE and Pool engines.
- The payload-out and final-out DMAs are pre-prepared SWDGE
  scatter-adds (onto zeroed DRAM) fired with trigger_dma: the
  descriptor-generation (625ns on the shared HWDGE) and DGE handoff
  (650ns) costs are paid off the critical path, at prep time.
"""

import numpy as np

import concourse.bass as bass
import concourse.mybir as mybir
import concourse.tile as tile
from concourse import bacc
from concourse.bass_utils import run_bass_kernel_spmd
from concourse.masks import make_identity
from concourse.tile_scheduler import dmasw_start_idx

B, N, D = 32, 32768, 128
NCORES = 8
SHARD = N // NCORES          # 4096
NJ = 4                       # n-groups per core (partition-packed)
NG = SHARD // NJ             # 1024 n per group
NH = 2                       # halves (PSUM banks) per group row: 2*512
HW = 512                     # half width

F32 = mybir.dt.float32
F16 = mybir.dt.float16
BF16 = mybir.dt.bfloat16
I16 = mybir.dt.int16

DTW = 32 + SHARD             # 4128: w16 [128,32] + 8 chunks of 512
# dt piece boundaries (cols of dtw): A=[0,1056) B=[1056,2080) C=[2080,3104) D=[3104,4128)
PA, PB, PC, PD = 1056, 2080, 3104, 4128

# cbc [12, CC] f16: 3 rows per j: (s2h, rh), (s2l, rh), (s2h, rl)
O_R = 0            # R12[3j+t, h, x]  [12, 2, 512]
O_SR = 1024        # Sr12[3j+t, 32j+b] (block diag) [12, 128]
O_ONE1 = 1152      # ones f16
CC = 1156

# cbb [32, CB] f32
O_ISC = 0          # inputs/sqrt(var) [32, 128]
O_C2 = 128         # -sqrt(a)/sqrt(var) [32, 1]
CB = 132

NQ = SHARD // 128  # 32 naug chunks
NW = 65            # f32 words per naug chunk row (130 bf16: 128 data, 1, pad)
FW = 130           # payload f32 cols actually used: [x0(128) | s | -M_core]
FWP = 192          # padded payload row (scatter elem 768B, %256)

import os as _os
N_FILL = int(_os.environ.get("K_N_FILL", "20"))   # PE warm-up fillers
W_FILL = int(_os.environ.get("K_W_FILL", "128"))  # filler output free size
USE_TRIG = int(_os.environ.get("K_TRIG", "1"))    # prepared scatter+trigger

_CACHE = {}

_STAGES = ["mm1", "exp", "et", "mm2", "agr", "full"]


def _build(with_collective=True, stage="full"):
    sidx = _STAGES.index(stage)
    nc = bacc.Bacc("TRN2", target_bir_lowering=False, debug=False,
                   num_devices=NCORES)

    dtw_d = nc.dram_tensor("dtw", [128, DTW], F16, kind="ExternalInput")
    cbc_d = nc.dram_tensor("cbc", [12, CC], F16, kind="ExternalInput")
    naug_d = nc.dram_tensor("naug", [128, NQ, NW], F32, kind="ExternalInput")
    cbb_d = nc.dram_tensor("cbb", [B, CB], F32, kind="ExternalInput")
    sidx_d = nc.dram_tensor("sidx", [128, 8], I16, kind="ExternalInput")

    out_d = nc.dram_tensor("out", [B, D], F32, kind="ExternalOutput")

    ag_in = nc.dram_tensor("ag_in_b", [B, FWP], F32)
    ag_out = nc.dram_tensor("ag_out_b", [B * NCORES, FWP], F32,
                            addr_space="Shared")

    with tile.TileContext(nc) as tc:
        with (
            tc.tile_pool(name="sb", bufs=1) as sb,
            tc.tile_pool(name="ps_l", bufs=1, space="PSUM") as ps_l,
            tc.tile_pool(name="ps_et", bufs=2, space="PSUM") as ps_et,
            tc.tile_pool(name="ps_x1", bufs=1, space="PSUM") as ps_x1,
            tc.tile_pool(name="ps_x2", bufs=1, space="PSUM") as ps_x2,
            tc.tile_pool(name="ps_a", bufs=1, space="PSUM") as ps_a,
        ):
            # ---- t=0 setup: filler weights + Exp-table warm-up ----
            wfill = sb.tile([128, 128], F16)
            nc.vector.memset(wfill, 0.0)
            warm = sb.tile([1, 2], F32)
            nc.vector.memset(warm[:, 0:1], 0.0)
            nc.scalar.activation(warm[:, 1:2], warm[:, 0:1],
                                 mybir.ActivationFunctionType.Exp,
                                 bias=warm[:, 0:1])

            # ---- input DMAs ----
            # cbc's descriptors generate on the Pool/SWDGE path, in
            # parallel with the SP/HWDGE descriptors of the dt pieces.
            cbc = sb.tile([12, CC], F16)
            nc.gpsimd.dma_start(out=cbc, in_=cbc_d.ap())
            dtw = sb.tile([128, DTW], F16)
            nc.sync.dma_start(out=dtw[:, 0:PA], in_=dtw_d.ap()[:, 0:PA])
            nc.sync.dma_start(out=dtw[:, PA:PB], in_=dtw_d.ap()[:, PA:PB])
            nc.sync.dma_start(out=dtw[:, PB:PC], in_=dtw_d.ap()[:, PB:PC])
            nc.sync.dma_start(out=dtw[:, PC:PD], in_=dtw_d.ap()[:, PC:PD])
            # scatter idx table + zero-fills ride before naug (tiny)
            sidxs = sb.tile([128, 8], I16)
            nc.sync.dma_start(out=sidxs, in_=sidx_d.ap())
            zrow = sb.tile([B, FWP], F32)
            nc.vector.memset(zrow, 0.0)
            nc.sync.dma_start(out=ag_in.ap(), in_=zrow)
            nc.sync.dma_start(out=out_d.ap(), in_=zrow[:, 0:D])
            naug = sb.tile([128, NQ, NW], F32)
            for q in range(2):
                nc.sync.dma_start(
                    out=naug[:, q * (NQ // 2):(q + 1) * (NQ // 2), :],
                    in_=naug_d.ap()[:, q * (NQ // 2):(q + 1) * (NQ // 2), :],
                )
            cbb = sb.tile([B, CB], F32)
            nc.sync.dma_start(out=cbb, in_=cbb_d.ap())
            nbf = naug.bitcast(BF16)       # [128, NQ, 2*NW]

            # scatter sources: zero the pad/unused regions once so the
            # full-rectangle scatter reads are defined
            agi = sb.tile([128, FWP], F32)
            nc.vector.memset(agi, 0.0)
            outt = sb.tile([128, D], F32)
            nc.vector.memset(outt, 0.0)

            # next free Tile DMASW lane (cbc's Pool DMACopy took lane 0)
            swlane0 = [1]

            # ---- on-chip constants (Pool, after DMA desc-gen) ----
            identh = sb.tile([128, 128], F16)
            make_identity(nc, identh)
            identf = sb.tile([128, 128], F32)
            nc.gpsimd.tensor_copy(identf, identh)


            w16 = dtw[:, 0:32]

            def dchunk(q):
                return dtw[:, 32 + 512 * q:32 + 512 * (q + 1)]

            inputs_sc = cbb[:, O_ISC:O_ISC + D]
            c2neg = cbb[:, O_C2:O_C2 + 1]
            r12 = cbc[:, O_R:O_R + NH * HW]                  # [12, 1024]
            sr12 = cbc[:, O_SR:O_SR + 128]                   # [12, 128]
            one16 = cbc[0:1, O_ONE1:O_ONE1 + 1]              # [1, 1] f16

            # shared 1-bank PSUM scratch: fillers, nm1 transpose, f_row
            # broadcast, and the [32,1] -M spread (writes are ordered)
            aux_ps = ps_a.tile([128, 128], F32)

            # ---- PE p-state warm-up fillers (tiny matmuls on wfill) ----
            for k in range(N_FILL):
                nc.tensor.matmul(aux_ps[0:16, 0:W_FILL], wfill[:, 0:16],
                                 wfill[:, 0:W_FILL], start=True, stop=True)

            # ---- mm1 l_0: j matmuls then sr12 closes the bank ----
            l0 = ps_l.tile([128, HW], F32, tag="l0")
            l1 = ps_l.tile([128, HW], F32, tag="l1")
            for j in range(NJ):
                nc.tensor.matmul(l0[32 * j:32 * j + 32, :], w16, dchunk(j),
                                 start=True, stop=False,
                                 tile_position=(0, 32 * j),
                                 skip_group_check=True)
            nc.tensor.matmul(l0, sr12, r12[:, 0:HW],
                             start=False, stop=True, tile_position=(0, 0),
                             skip_group_check=True)

            nmh = sb.tile([128, 2], F32)
            nc.vector.tensor_reduce(nmh[:, 0:1], l0,
                                    axis=mybir.AxisListType.X,
                                    op=mybir.AluOpType.max, negate=True)

            # ---- mm1 l_1: sr12 first so the last data matmul closes ----
            nc.tensor.matmul(l1, sr12, r12[:, HW:2 * HW],
                             start=True, stop=False, tile_position=(0, 0),
                             skip_group_check=True)
            for j in range(NJ):
                nc.tensor.matmul(l1[32 * j:32 * j + 32, :], w16,
                                 dchunk(4 + j), start=False, stop=True,
                                 tile_position=(0, 32 * j),
                                 skip_group_check=True)

            if sidx >= 1:
                nc.vector.tensor_reduce(nmh[:, 1:2], l1,
                                        axis=mybir.AxisListType.X,
                                        op=mybir.AluOpType.max, negate=True)
                nm1 = sb.tile([128, 1], F32)
                nc.vector.tensor_reduce(nm1, nmh, axis=mybir.AxisListType.X,
                                        op=mybir.AluOpType.min)
                # exp in the LOCAL per-partition frame; h1 in two pieces
                # so its E^T transposes can start earlier
                e_sb = sb.tile([128, NG], F16)
                nc.scalar.activation(e_sb[:, 0:HW], l0,
                                     mybir.ActivationFunctionType.Exp,
                                     bias=nm1)
                nc.scalar.activation(e_sb[:, HW:2 * HW], l1,
                                     mybir.ActivationFunctionType.Exp,
                                     bias=nm1)

                # ---- off-critical: f_j[b] = exp(m_p - M_core(b)) per
                # j-group as a [32, 4] tile + the -M payload column ----
                nc.tensor.transpose(aux_ps[0:1, :], nm1, identf)
                nmc = sb.tile([1, B], F16)     # -M_core per b
                nc.vector.tensor_reduce(
                    nmc,
                    aux_ps[0:1, :].rearrange("p (j b) -> p b j", j=NJ),
                    axis=mybir.AxisListType.X, op=mybir.AluOpType.min)
                drow = sb.tile([1, 128], F16)  # m_p - M_core(b(p))
                nc.vector.tensor_tensor(
                    drow.rearrange("p (j b) -> p j b", j=NJ),
                    nmc[:, None, :].broadcast_to([1, NJ, B]),
                    aux_ps[0:1, :].rearrange("p (j b) -> p j b", j=NJ),
                    op=mybir.AluOpType.subtract)

            if sidx >= 2:
                # ---- E^T tiles: transpose pairs into one PSUM tile, then
                # one 256-wide DVE copy per pair ----
                et_sb = sb.tile([128, NG // 128, 128], F16)
                for cp in range(NG // 256):
                    etp = ps_et.tile([128, 256], F16, tag="et")
                    for i in range(2):
                        c = 2 * cp + i
                        nc.tensor.transpose(
                            etp[:, 128 * i:128 * i + 128],
                            e_sb[:, 128 * c:128 * c + 128], identh)
                    nc.vector.tensor_copy(
                        et_sb[:, 2 * cp:2 * cp + 2, :],
                        etp.rearrange("p (c w) -> p c w", c=2))

            if sidx >= 1:
                # frow emitted after the exps so Tile keeps the Act queue
                # order (exp h0, h1a, h1b, frow)
                frow = sb.tile([1, 128], F16)
                nc.scalar.activation(frow, drow,
                                     mybir.ActivationFunctionType.Exp)
                # f-spread matmuls: aux cols 1..4 <- f_j, col 5 <- -M
                for j in range(NJ):
                    nc.tensor.matmul(aux_ps[0:B, 1 + j:2 + j],
                                     frow[:, 32 * j:32 * j + 32], one16,
                                     start=True, stop=True)
                nc.tensor.matmul(aux_ps[0:B, 5:6], nmc, one16,
                                 start=True, stop=True)
                fqm = sb.tile([B, 5], F32)     # [f_0..f_3 | -M]
                nc.vector.tensor_copy(fqm, aux_ps[0:B, 1:6])

            if sidx >= 3:
                # ---- mm2: per-j [x0 | s] tiles ----
                # separate pools: readers of xt23 (the Act scales) start
                # as soon as ITS writers finish (reader deps are
                # pool-granular), not when xt01's last matmul lands
                xt01 = ps_x1.tile([B, 2, D + 1], F32, tag="x01")
                xt23 = ps_x2.tile([B, 2, D + 1], F32, tag="x23")
                x_j = [xt01[:, 0, :], xt01[:, 1, :],
                       xt23[:, 0, :], xt23[:, 1, :]]
                # j=2,3 first: their Act-scaled combine terms are ready
                # early, so only one DVE op trails the last matmul (j=1)
                for j in (2, 3, 0, 1):
                    for c in range(NG // 128):
                        nc.tensor.matmul(
                            x_j[j], et_sb[:, c, 32 * j:32 * j + 32],
                            nbf[:, (NG // 128) * j + c, 0:D + 1],
                            start=(c == 0), stop=(c == NG // 128 - 1),
                        )

                # ---- payload [x0 | s | -M]: j-tree across DVE + Pool ----
                nc.vector.tensor_copy(agi[0:B, D + 1:D + 2], fqm[:, 4:5])
                # u2/u3 first on DVE so the Pool fold t23 (the trigger's
                # queue gate, with the payload prep behind it) starts as
                # early as possible; u0/t01 then overlap the prep desc-gen
                u2 = sb.tile([B, D + 1], F32)
                nc.vector.tensor_scalar_mul(u2, x_j[2], fqm[:, 2:3])
                u3 = sb.tile([B, D + 1], F32)
                nc.vector.tensor_scalar_mul(u3, x_j[3], fqm[:, 3:4])
                t23 = sb.tile([B, D + 1], F32)
                nc.gpsimd.tensor_tensor(t23, u2, u3,
                                        op=mybir.AluOpType.add)
                u0 = sb.tile([B, D + 1], F32)
                nc.vector.tensor_scalar_mul(u0, x_j[0], fqm[:, 0:1])
                t01 = sb.tile([B, D + 1], F32)
                nc.vector.scalar_tensor_tensor(
                    t01, x_j[1], fqm[:, 1:2], u0,
                    op0=mybir.AluOpType.mult, op1=mybir.AluOpType.add)
                nc.vector.tensor_tensor(agi[0:B, 0:D + 1], t01, t23,
                                        op=mybir.AluOpType.add)
                # prepared SWDGE scatter (onto the zeroed ag_in), fired by
                # trigger_dma once agi is ready: no HWDGE/DGE latency here.
                # sem must be the Tile DMASW lane sem for this prep: Pool
                # DMA insts take lanes in emission order (cbc=0, this=1).
                if USE_TRIG in (1, 3):
                    nc.gpsimd.dma_scatter_add(
                        ag_in.ap(), agi[:, None, :], sidxs[:, 0:2], B, B,
                        FWP, prepare_only=True,
                        sem=tc.sems[dmasw_start_idx + swlane0[0]])
                    swlane0[0] += 1
                    nc.gpsimd.trigger_dma(count=None)
                else:
                    nc.sync.dma_start(out=ag_in.ap()[:, 0:FW],
                                      in_=agi[0:B, 0:FW])

            if sidx >= 4:
                agg = sb.tile([B, NCORES, FW], F32)
                if with_collective:
                    nc.gpsimd.collective_compute(
                        "AllGather",
                        mybir.AluOpType.bypass,
                        replica_groups=[list(range(NCORES))],
                        ins=[ag_in.ap().opt()],
                        outs=[ag_out.ap().opt()],
                    )
                    nc.sync.dma_start(
                        out=agg,
                        in_=ag_out.ap().rearrange("(c p) f -> p c f",
                                                  p=B)[:, :, 0:FW],
                    )
                else:
                    # timing-sim stand-in (collective itself not modeled):
                    # same payload store + a broadcast read-back
                    nc.sync.dma_start(
                        out=agg,
                        in_=ag_in.ap()[:, None, 0:FW].broadcast_to(
                            [B, NCORES, FW]),
                    )

            if sidx >= 5:
                # ---- cross-core combine (identical on every core) ----
                nmg8 = agg[:, :, D + 1]          # [32, 8] strided view
                nmming = sb.tile([B, 1], F32)    # = -M_global
                nc.vector.tensor_reduce(nmming, nmg8,
                                        axis=mybir.AxisListType.X,
                                        op=mybir.AluOpType.min)
                fg = sb.tile([B, NCORES], F32)   # exp(M_c - M_global)
                nc.scalar.activation(fg, nmg8,
                                     mybir.ActivationFunctionType.Exp,
                                     bias=nmming, scale=-1.0)
                # two accumulator chains: DVE scalar-ptr ops for c0..4;
                # c5..7 scale on Act (Copy + per-partition scale) with
                # SBUF-only TT folds on Pool (Pool has no TensorScalarPtr
                # and cannot read PSUM)
                accv = sb.tile([B, D + 1], F32)
                nc.vector.tensor_scalar_mul(accv, agg[:, 0, 0:D + 1],
                                            fg[:, 0:1])
                for c in range(1, 5):
                    nc.vector.scalar_tensor_tensor(
                        accv, agg[:, c, 0:D + 1], fg[:, c:c + 1], accv,
                        op0=mybir.AluOpType.mult, op1=mybir.AluOpType.add)
                u567 = []
                for c in range(5, 8):
                    uc = sb.tile([B, D + 1], F32, name=f"u{c}")
                    u567.append(uc)
                for i, c in enumerate(range(5, 8)):
                    nc.scalar.activation(u567[i], agg[:, c, 0:D + 1],
                                         mybir.ActivationFunctionType.Copy,
                                         scale=fg[:, c:c + 1])
                # single Pool fold (t56) so the out-prep behind it on the
                # Pool queue starts ~0.4us earlier; the rest folds on DVE
                t56 = sb.tile([B, D + 1], F32)
                nc.gpsimd.tensor_tensor(t56, u567[0], u567[1],
                                        op=mybir.AluOpType.add)
                accg1 = sb.tile([B, D + 1], F32)
                nc.vector.tensor_tensor(accg1, accv, t56,
                                        op=mybir.AluOpType.add)
                accg = sb.tile([B, D + 1], F32)
                nc.vector.tensor_tensor(accg, accg1, u567[2],
                                        op=mybir.AluOpType.add)

                # ---- final: out = x0_tot * (c2neg/s_tot) + inputs_sc ----
                rec = sb.tile([B, 1], F32)
                nc.vector.reciprocal(rec, accg[:, D:D + 1])
                c2r = sb.tile([B, 1], F32)
                nc.vector.tensor_tensor(c2r, rec, c2neg,
                                        op=mybir.AluOpType.mult)
                nc.vector.scalar_tensor_tensor(
                    outt[0:B, :], accg[:, 0:D], c2r, inputs_sc,
                    op0=mybir.AluOpType.mult, op1=mybir.AluOpType.add,
                )
                # fire the prepared out-scatter (prep here so its outt
                # read-dep defers to this trigger)
                if USE_TRIG == 1:
                    nc.gpsimd.dma_scatter_add(
                        out_d.ap(), outt[:, None, :], sidxs[:, 0:2], B, B,
                        D, prepare_only=True,
                        sem=tc.sems[dmasw_start_idx + swlane0[0]])
                    swlane0[0] += 1
                    nc.gpsimd.trigger_dma(count=None)
                elif USE_TRIG == 2:
                    nc.gpsimd.dma_scatter_add(
                        out_d.ap(), outt[:, None, :], sidxs[:, 0:2], B, B,
                        D, prepare_only=True,
                        sem=tc.sems[dmasw_start_idx + swlane0[0]])
                    swlane0[0] += 1
                    nc.gpsimd.trigger_dma(count=None)
                else:
                    nc.sync.dma_start(out=out_d.ap(), in_=outt[0:B, :])

    nc.compile()
    return nc


def _get_nc():
    if "nc" not in _CACHE:
        _CACHE["nc"] = _build()
    return _CACHE["nc"]


def _prepare_in_maps(inputs, alphas, data_batch):
    import ml_dtypes

    inputs = np.asarray(inputs, np.float32)
    alphas = np.asarray(alphas, np.float32)
    data = np.ascontiguousarray(np.asarray(data_batch, np.float32))

    var = 1.0 - alphas
    s1 = np.sqrt(alphas) / var                        # [B]
    s2 = -alphas / (2.0 * var)                        # [B]
    w_all = (inputs * s1[:, None]).T.astype(np.float16)   # [D, B] f16
    inputs_sc = (inputs / np.sqrt(var)[:, None]).astype(np.float32)
    c2neg = (-np.sqrt(alphas) / np.sqrt(var)).astype(np.float32)

    dataT = np.ascontiguousarray(data.T)              # [D, N]
    r = (data * data).sum(axis=1).astype(np.float32)  # [N]
    r_h = r.astype(np.float16)
    r_l = (r - r_h.astype(np.float32)).astype(np.float16)
    s2_h = s2.astype(np.float16)
    s2_l = (s2 - s2_h.astype(np.float32)).astype(np.float16)

    cbb = np.zeros((B, CB), np.float32)
    cbb[:, O_ISC:O_ISC + D] = inputs_sc
    cbb[:, O_C2] = c2neg

    # scatter idx table [128, 8]: value (p%16 + 16x) if < 32 else -1,
    # 16-row-wrapped and replicated to every 16-partition group
    k = (np.arange(16)[:, None] + 16 * np.arange(8)[None, :]).astype(np.int16)
    k = np.where(k < B, k, np.int16(-1))
    sidx = np.tile(k, (8, 1))

    # Sr12: block-diag rows (s2h, s2l, s2h) per j
    sr12 = np.zeros((12, 128), np.float16)
    for j in range(NJ):
        sr12[3 * j + 0, 32 * j:32 * j + 32] = s2_h
        sr12[3 * j + 1, 32 * j:32 * j + 32] = s2_l
        sr12[3 * j + 2, 32 * j:32 * j + 32] = s2_h

    in_maps = []
    for cid in range(NCORES):
        lo = cid * SHARD
        dt_c = dataT[:, lo:lo + SHARD].astype(np.float16)  # [128, 4096]

        # dtw = [w16 | chunk (h, j) = dataT cols 1024j + 512h + x]
        dtw = np.empty((128, DTW), np.float16)
        dtw[:, 0:32] = w_all
        for h in range(NH):
            for j in range(NJ):
                q = 4 * h + j
                dtw[:, 32 + 512 * q:32 + 512 * (q + 1)] = \
                    dt_c[:, 1024 * j + 512 * h:1024 * j + 512 * h + HW]

        # R12 rows per j: (rh, rh, rl)
        cbc = np.zeros((12, CC), np.float16)
        for j in range(NJ):
            for h in range(NH):
                sl = slice(lo + 1024 * j + 512 * h,
                           lo + 1024 * j + 512 * h + HW)
                cbc[3 * j + 0, O_R + h * HW:O_R + (h + 1) * HW] = r_h[sl]
                cbc[3 * j + 1, O_R + h * HW:O_R + (h + 1) * HW] = r_h[sl]
                cbc[3 * j + 2, O_R + h * HW:O_R + (h + 1) * HW] = r_l[sl]
        cbc[:, O_SR:O_SR + 128] = sr12
        cbc[:, O_ONE1] = 1.0

        # naug chunks: [128 rows, 130 bf16] = [data | 1.0 | 0]
        nrows = np.zeros((SHARD, 2 * NW), ml_dtypes.bfloat16)
        nrows[:, 0:D] = data[lo:lo + SHARD].astype(ml_dtypes.bfloat16)
        nrows[:, D] = 1.0
        naug = np.ascontiguousarray(
            nrows.reshape(NQ, 128, 2 * NW).transpose(1, 0, 2)
        ).view(np.uint16).view(np.float32)            # [128, NQ, NW]

        in_maps.append({
            "dtw": dtw,
            "cbc": cbc,
            "naug": naug,
            "cbb": cbb,
            "sidx": sidx,
        })
    return in_maps


def run(inputs, alphas, data_batch, trace=False, trace_kwargs=None):
    nc = _get_nc()
    in_maps = _prepare_in_maps(inputs, alphas, data_batch)
    res = run_bass_kernel_spmd(
        nc, in_maps, core_ids=list(range(NCORES)),
        trace=trace, **(trace_kwargs or {}),
    )
    return res.results[0]["out"].astype(np.float32), res


def kernel(inputs, alphas, data_batch):
    out, _ = run(inputs, alphas, data_batch)
    return out
